# revision 31
# baseline (speedup 1.0000x reference)
"""MatchingNet head (cosine-sim kNN aggregation) on 8 trn2 NeuronCores.

Reference computation:
    sim[m, n] = <fX[m], gS[n]> / max(||fX[m]|| * ||gS[n]||, 1e-8)
    out[m, c] = sum_n sim[m, n] * onehot(trainTarget)[n, c]

Exact algebraic reassociation (the eps guard never binds for D=1024 randn
rows, whose norms concentrate around 32):
    A = gS.T @ (onehot / ||gS||)           # [D, C]
    out = diag(1/||fX||) @ (fX @ A)        # [M, C]

Two SPMD launches (cross-core collectives measured ~85us for 256KB on this
runtime -- far slower than a host round-trip between launches):
  Phase 1: gS row-sharded; core i computes the partial
           A_i = (onehot_i / ||gS_i||).T @ gS_i  in [C, D] layout over its
           512 supports, AND the inverse query norms 1/||fX|| for its own
           fX shard (from a row-major fp8 copy -- phase 1 has idle
           DVE/Scalar area, phase 2 doesn't).  The host sums the eight
           A partials and relayouts (0.5 MFLOP + tiny transposes).
  Phase 2: fX row-sharded; each core computes its output slab transposed,
           [C, M/8], from the replicated A.T, scales by the precomputed
           rinv row broadcast via two K=1 matmuls, and stores; the host
           transposes back during the gather.

All heavy streams are bf16 (host-cast): halves both DMA bytes and PE
passes (fp32 matmuls are 2-pass on trn2).  Norm reductions ride the
free-dim accumulator of one elementwise pass per 128-row tile; elementwise
throughput is ~1 elem/cycle/lane regardless of dtype, so the 12 norm ops
are split across DVE (scalar_tensor_tensor) and Scalar (Square+accum) and
overlap the DMA/matmul streams.  The fp8 copy of fX only feeds ||fX||^2
(quantization error ~0.1% there); the dot products use bf16.
"""

import numpy as np
from contextlib import ExitStack

import concourse.bass as bass  # noqa: F401
import concourse.tile as tile
import concourse.mybir as mybir
from concourse import bacc, bass2jax
from concourse.bass_utils import run_bass_kernel_spmd

N, D, C, M = 4096, 1024, 64, 8192
NCORES = 8
NS = N // NCORES   # 512 supports per core (phase 1)
MS = M // NCORES   # 1024 queries per core (phase 2)
P = 128
NT = NS // P       # 4 n-tiles per core
DC = D // P        # 8 d-chunks / 8 query-tiles per core
HB = 512           # half of MS / PSUM-bank width in fp32
F32 = mybir.dt.float32
BF16 = mybir.dt.bfloat16
FXR_DT = mybir.dt.float8e4
NPBF16 = mybir.dt.np(BF16)
NPFXR = mybir.dt.np(FXR_DT)
AF = mybir.ActivationFunctionType
MUL = mybir.AluOpType.mult

_CACHE = {}


def _build_phase1():
    nc = bacc.Bacc(
        "TRN2", target_bir_lowering=False, debug=False, num_devices=NCORES
    )
    gs = nc.dram_tensor("gs", [P, NT, D], BF16, kind="ExternalInput").ap()
    oh = nc.dram_tensor("oh", [P, NT, C], BF16, kind="ExternalInput").ap()
    fxr = nc.dram_tensor("fxr", [P, DC, D], FXR_DT, kind="ExternalInput").ap()
    atp = nc.dram_tensor("atp", [C, D], F32, kind="ExternalOutput").ap()
    rinv = nc.dram_tensor("rinv", [P, DC], BF16, kind="ExternalOutput").ap()

    with tile.TileContext(nc) as tc, ExitStack() as ctx:
        const_pool = ctx.enter_context(tc.tile_pool(name="const", bufs=1))
        sq_pool = ctx.enter_context(tc.tile_pool(name="sqp", bufs=2))
        st_pool = ctx.enter_context(tc.tile_pool(name="stp", bufs=4))
        w_pool = ctx.enter_context(tc.tile_pool(name="wp", bufs=2))
        os_pool = ctx.enter_context(tc.tile_pool(name="osp", bufs=1))
        psA = ctx.enter_context(tc.tile_pool(name="psA", bufs=1, space="PSUM"))

        # First scalar ops are a Sqrt and a Square so bacc hoists BOTH
        # act-table loads (they live in different sets) to kernel start.
        dumm = st_pool.tile([P, 1], F32, tag="dumm")
        nc.gpsimd.memset(dumm[:], 1.0)
        dumm2 = st_pool.tile([P, 1], F32, tag="dumm2")
        nc.scalar.activation(dumm2[:], dumm[:], AF.Sqrt)
        dumm3 = st_pool.tile([P, 1], F32, tag="dumm3")
        nc.scalar.activation(dumm3[:], dumm[:], AF.Square)

        # Single DMA ring (sync): concurrent rings delay each other's
        # completion semaphores by several us on this part.  gs tiles
        # interleave with paired fxr tiles so both norm chains stream.
        oh_sb = const_pool.tile([P, NT * C], BF16, tag="oh")
        gs_sb = const_pool.tile([P, NT * D], BF16, tag="gs")
        fxr_sb = const_pool.tile([P, DC * D], FXR_DT, tag="fxr")
        fxr_v = fxr_sb[:].rearrange("p (t d) -> p t d", t=DC)
        # All gs tiles first: the gs->wt chains gate the matmul stream.
        # oh (64KB, first needed by wt0 at ~14.5us) rides after them, and
        # the fxr pairs (gating only the tiny independent rinv out) last.
        for t in range(NT):
            nc.sync.dma_start(gs_sb[:, t * D:(t + 1) * D], gs[:, t, :])
        nc.sync.dma_start(
            oh_sb[:].rearrange("p (t c) -> p t c", t=NT), oh[:, :, :]
        )
        for t in range(NT):
            nc.sync.dma_start(
                fxr_v[:, 2 * t:2 * t + 2, :], fxr[:, 2 * t:2 * t + 2, :]
            )

        # Support norms + weighted one-hot, tiles alternating DVE/Scalar so
        # the matmul stream is paced by two chains instead of one.
        pa = [
            psA.tile([C, HB], F32, tag=f"at{h}", name=f"pa{h}")
            for h in range(2)
        ]
        # High priority: the scheduler must not let the fxr norm ops below
        # delay these matmul-gating chains on the saturated DVE/Scalar.
        grinv_last = None
        with tc.high_priority():
            for t in range(NT):
                seg = gs_sb[:, t * D:(t + 1) * D]
                gsq = st_pool.tile([P, 1], F32, tag=f"gsq{t}")
                sqt = sq_pool.tile([P, D], BF16, tag=f"sq{t % 2}")
                if t % 2 == 0:
                    nc.vector.scalar_tensor_tensor(
                        out=sqt[:], in0=seg, scalar=1.0, in1=seg,
                        op0=MUL, op1=MUL, accum_out=gsq[:],
                    )
                else:
                    nc.scalar.activation(
                        sqt[:], seg, AF.Square, accum_out=gsq[:]
                    )
                gnorm = st_pool.tile([P, 1], F32, tag=f"gn{t}")
                nc.scalar.activation(gnorm[:], gsq[:], AF.Sqrt)
                grinv = st_pool.tile([P, 1], F32, tag=f"gr{t}")
                nc.vector.reciprocal(grinv[:], gnorm[:])
                grinv_last = grinv
                wt = w_pool.tile([P, C], BF16, tag=f"w{t % 2}")
                nc.vector.tensor_scalar_mul(
                    wt[:], oh_sb[:, t * C:(t + 1) * C], grinv[:]
                )
                for h in range(2):
                    nc.tensor.matmul(
                        pa[h][:],
                        wt[:],
                        seg[:, h * HB:(h + 1) * HB],
                        start=(t == 0),
                        stop=(t == NT - 1),
                    )

        # Query norms for this core's fX shard: one fused square+row-sum
        # per 128-query tile, DVE tiles 0-3 (stt), Scalar tiles 4-7
        # (Square+accum; Scalar also carries the gs squares above and the
        # sqrt chain below).
        # DVE takes 5 tiles, Scalar 3: Scalar's per-op cost is higher
        # (+0.28us ACTIVATION_READ_ACCUMULATOR) and it also carries the
        # sqrt chain, so this evens the two engines' finish times.
        nsq = st_pool.tile([P, DC], F32, tag="nsq")
        for t in range(DC):
            sqf = sq_pool.tile([P, D], BF16, tag=f"sqf{t % 2}")
            if t < 5:
                # op0=bypass ignores the scalar numerically, but reading the
                # last gs reciprocal makes these DVE ops *depend* on the
                # matmul-gating chains, so the in-order DVE queue cannot
                # stall a gs chain behind a 1.2us fxr op.
                nc.vector.scalar_tensor_tensor(
                    out=sqf[:], in0=fxr_v[:, t, :],
                    scalar=grinv_last[:],
                    in1=fxr_v[:, t, :],
                    op0=mybir.AluOpType.bypass, op1=MUL,
                    accum_out=nsq[:, t:t + 1],
                )
            else:
                nc.scalar.activation(
                    sqf[:], fxr_v[:, t, :], AF.Square,
                    accum_out=nsq[:, t:t + 1],
                )
        nrm = st_pool.tile([P, DC], F32, tag="nrm")
        nc.scalar.activation(nrm[:], nsq[:], AF.Sqrt)
        rinv_sb = st_pool.tile([P, DC], BF16, tag="rinv")
        with nc.allow_low_precision(
            reason="bf16 1/||fX||; rel-err budget is 2e-2"
        ):
            nc.vector.reciprocal(rinv_sb[:], nrm[:])
        nc.sync.dma_start(rinv[:, :], rinv_sb[:])

        # Drain PSUM on two engines, store on the (warm) sync ring.  High
        # priority so the drains preempt any remaining fxr norm ops.
        asb = os_pool.tile([C, D], F32, tag="asb")
        with tc.high_priority():
            nc.scalar.copy(asb[:, 0:HB], pa[0][:])
            nc.vector.tensor_copy(asb[:, HB:D], pa[1][:])
            nc.sync.dma_start(atp[:, 0:HB], asb[:, 0:HB])
            nc.sync.dma_start(atp[:, HB:D], asb[:, HB:D])

    nc.compile()
    return nc


def _build_phase2():
    nc = bacc.Bacc(
        "TRN2", target_bir_lowering=False, debug=False, num_devices=NCORES
    )
    at = nc.dram_tensor("at", [P, DC, C], BF16, kind="ExternalInput").ap()
    fxt = nc.dram_tensor("fxt", [P, DC, MS], BF16, kind="ExternalInput").ap()
    rinvr = nc.dram_tensor("rinvr", [1, MS], BF16, kind="ExternalInput").ap()
    outT = nc.dram_tensor("outT", [C, MS], F32, kind="ExternalOutput").ap()

    with tile.TileContext(nc) as tc, ExitStack() as ctx:
        const_pool = ctx.enter_context(tc.tile_pool(name="const", bufs=1))
        st_pool = ctx.enter_context(tc.tile_pool(name="stp", bufs=2))
        os_pool = ctx.enter_context(tc.tile_pool(name="osp", bufs=1))
        psO = ctx.enter_context(tc.tile_pool(name="psO", bufs=1, space="PSUM"))
        psB = ctx.enter_context(tc.tile_pool(name="psB", bufs=1, space="PSUM"))

        ones_c = const_pool.tile([1, C], BF16, tag="ones_c")
        nc.gpsimd.memset(ones_c[:], 1.0)

        # Single DMA ring (sync), coarse chunk-pair descriptors: issue
        # serialization on the sync engine (~0.6us per descriptor) and
        # per-descriptor completion lag were pacing the matmul stream.
        rv_sb = const_pool.tile([1, MS], BF16, tag="rv")
        nc.sync.dma_start(rv_sb[:], rinvr[:, :])
        at_sb = const_pool.tile([P, DC * C], BF16, tag="at")
        nc.sync.dma_start(
            at_sb[:].rearrange("p (k c) -> p k c", k=DC), at[:, :, :]
        )
        # h=0 m-halves as chunk pairs, then the h=1 halves: the half-0
        # scale/store runs while half 1 is still streaming.
        fxt_sb = const_pool.tile([P, DC * MS], BF16, tag="fxt")
        fxt_v = fxt_sb[:].rearrange("p (k m) -> p k m", k=DC)
        # Chunk-pair descriptors, except the tail of the h=1 stream stays
        # single-chunk: the final descriptor's completion semaphore (+2.4us
        # lag) gates the last matmul -> mult -> store chain, and a smaller
        # last transfer fires it sooner.
        groups = [(0, 2), (2, 4), (4, 6), (6, 8)]
        tail_groups = [(0, 2), (2, 4), (4, 6), (6, 7), (7, 8)]
        for h, gl in ((0, groups), (1, tail_groups)):
            for a, b in gl:
                nc.sync.dma_start(
                    fxt_v[:, a:b, h * HB:(h + 1) * HB],
                    fxt[:, a:b, h * HB:(h + 1) * HB],
                )

        # Broadcast rinv across the 64 class rows with two K=1 matmuls --
        # done up front, before the main stream needs the PE.
        rb_sb = []
        for h in range(2):
            rb = psB.tile([C, HB], F32, tag=f"rb{h}", name=f"rb{h}")
            nc.tensor.matmul(
                rb[:], ones_c[:], rv_sb[0:1, h * HB:(h + 1) * HB],
                start=True, stop=True,
            )
            rbs = st_pool.tile([C, HB], F32, tag=f"rbs{h}")
            nc.scalar.copy(rbs[:], rb[:])
            rb_sb.append(rbs)

        po = [
            psO.tile([C, HB], F32, tag=f"po{h}", name=f"po{h}")
            for h in range(2)
        ]
        osb = os_pool.tile([C, MS], F32, tag="osb")
        for h in range(2):
            for k in range(DC):
                nc.tensor.matmul(
                    po[h][:],
                    at_sb[:, k * C:(k + 1) * C],
                    fxt_v[:, k, h * HB:(h + 1) * HB],
                    start=(k == 0),
                    stop=(k == DC - 1),
                )
            nc.vector.tensor_mul(
                osb[:, h * HB:(h + 1) * HB], po[h][:], rb_sb[h][:]
            )
            nc.sync.dma_start(outT[:, h * HB:(h + 1) * HB],
                              osb[:, h * HB:(h + 1) * HB])

    nc.compile()
    return nc


def _get_ncs():
    if "nc1" not in _CACHE:
        _CACHE["nc1"] = _build_phase1()
        _CACHE["nc2"] = _build_phase2()
    return _CACHE["nc1"], _CACHE["nc2"]


class _FakeResult:
    def __init__(self, results):
        self.results = results
        self.exec_time_ns = None
        self.instructions_and_trace = None


def _make_runner(nc):
    """One persistently-jitted shard_map executable for this Bass module.

    run_bass_via_pjrt rebuilds its jit closure per call, which retraces and
    re-lowers the HLO every invocation (~3 s/launch of host time). Caching
    the jitted callable keeps warmed kernel() calls fast; the device-side
    NEFF and its execution are identical.
    """
    import jax
    import numpy as _np

    bass2jax.install_neuronx_cc_hook()
    Mesh = bass2jax.Mesh
    PartitionSpec = bass2jax.PartitionSpec
    shard_map = bass2jax.shard_map

    partition_name = (
        nc.partition_id_tensor.name if nc.partition_id_tensor else None
    )
    in_names, out_names, out_avals, zero_shapes = [], [], [], []
    for alloc in nc.m.functions[0].allocations:
        if not isinstance(alloc, mybir.MemoryLocationSet):
            continue
        name = alloc.memorylocations[0].name
        if alloc.kind == "ExternalInput":
            if name != partition_name:
                in_names.append(name)
        elif alloc.kind == "ExternalOutput":
            shape = tuple(alloc.tensor_shape)
            dtype = mybir.dt.np(alloc.dtype)
            out_avals.append(jax.core.ShapedArray(shape, dtype))
            out_names.append(name)
            zero_shapes.append((shape, dtype))
    n_params = len(in_names)
    all_in = list(in_names) + list(out_names)
    if partition_name is not None:
        all_in.append(partition_name)
    donate = tuple(range(n_params, n_params + len(out_names)))

    def _body(*args):
        operands = list(args)
        if partition_name is not None:
            operands.append(bass2jax.partition_id_tensor())
        outs = bass2jax._bass_exec_p.bind(
            *operands,
            out_avals=tuple(out_avals),
            in_names=tuple(all_in),
            out_names=tuple(out_names),
            lowering_input_output_aliases=(),
            sim_require_finite=True,
            sim_require_nnan=True,
            nc=nc,
        )
        return tuple(outs)

    devices = jax.devices()[:NCORES]
    mesh = Mesh(_np.asarray(devices), ("core",))
    nspec = n_params + len(out_names)
    sharded = jax.jit(
        shard_map(
            _body, mesh=mesh,
            in_specs=(PartitionSpec("core"),) * nspec,
            out_specs=(PartitionSpec("core"),) * len(out_names),
            check_rep=False,
        ),
        donate_argnums=donate,
        keep_unused=True,
    )

    def runner(in_maps):
        concat_in = [
            _np.concatenate([_np.asarray(m[name]) for m in in_maps], axis=0)
            for name in in_names
        ]
        concat_zeros = [
            _np.zeros((NCORES * s[0], *s[1:]), dt) for s, dt in zero_shapes
        ]
        out_arrs = sharded(*concat_in, *concat_zeros)
        return _FakeResult([
            {
                name: _np.asarray(out_arrs[i]).reshape(
                    NCORES, *out_avals[i].shape
                )[c]
                for i, name in enumerate(out_names)
            }
            for c in range(NCORES)
        ])

    return runner


def _get_runners():
    if "run1" not in _CACHE:
        nc1, nc2 = _get_ncs()
        _CACHE["run1"] = _make_runner(nc1)
        _CACHE["run2"] = _make_runner(nc2)
    return _CACHE["run1"], _CACHE["run2"]


def _tile_rows(arr, ntiles):
    """[ntiles*128, F] -> [128, ntiles, F] with [p, t, f] = arr[t*128+p, f]."""
    f = arr.shape[1]
    return np.ascontiguousarray(arr.reshape(ntiles, P, f).transpose(1, 0, 2))


def run(gS, fX, trainTarget, nClasses, trace=False, **spmd_kwargs):
    nc1, nc2 = _get_ncs()
    gS = np.asarray(gS, dtype=np.float32)
    fX = np.asarray(fX, dtype=np.float32)
    tt = np.asarray(trainTarget).astype(np.int64).ravel()
    nc_classes = int(np.asarray(nClasses))
    assert nc_classes == C and gS.shape == (N, D) and fX.shape == (M, D)

    oh = np.zeros((N, C), dtype=NPBF16)
    oh[np.arange(N), tt] = 1.0
    gS_bf = gS.astype(NPBF16)

    in_maps1 = []
    for i in range(NCORES):
        gsl = gS_bf[i * NS:(i + 1) * NS]
        osl = oh[i * NS:(i + 1) * NS]
        fxl = fX[i * MS:(i + 1) * MS]
        in_maps1.append({
            "gs": _tile_rows(gsl, NT),
            "oh": _tile_rows(osl, NT),
            "fxr": _tile_rows(fxl, DC).astype(NPFXR),
        })
    if trace or spmd_kwargs:
        res1 = run_bass_kernel_spmd(
            nc1, in_maps1, core_ids=list(range(NCORES)), trace=trace,
            **spmd_kwargs
        )
    else:
        res1 = _get_runners()[0](in_maps1)
    # gather-reduce the partial A's [C, D]; retile A.T to [128, 8, 64] bf16
    a_full = np.zeros((C, D), dtype=np.float32)
    for i in range(NCORES):
        a_full += res1.results[i]["atp"]
    at_tiled = np.ascontiguousarray(
        a_full.T.reshape(DC, P, C).transpose(1, 0, 2)
    ).astype(NPBF16)

    in_maps2 = []
    for i in range(NCORES):
        sl = fX[i * MS:(i + 1) * MS]                      # [MS, D]
        fxt_tiled = np.ascontiguousarray(
            sl.T.reshape(DC, P, MS).transpose(1, 0, 2)
        ).astype(NPBF16)
        rinvr = np.ascontiguousarray(
            res1.results[i]["rinv"].T
        ).reshape(1, MS)
        in_maps2.append({"at": at_tiled, "fxt": fxt_tiled, "rinvr": rinvr})
    if trace or spmd_kwargs:
        res2 = run_bass_kernel_spmd(
            nc2, in_maps2, core_ids=list(range(NCORES)), trace=trace,
            **spmd_kwargs
        )
    else:
        res2 = _get_runners()[1](in_maps2)
    outs = [
        np.ascontiguousarray(res2.results[i]["outT"].T)
        for i in range(NCORES)
    ]
    full = np.concatenate(outs, axis=0)
    return full, (res1, res2)


def kernel(gS, fX, trainTarget, nClasses):
    full, _ = run(gS, fX, trainTarget, nClasses)
    return full
